# revision 12
# baseline (speedup 1.0000x reference)
"""Trainium2 Bass kernel for nn_Encoder_85942295593405 (GNN message passing).

Math (reference):
  emb  = spikes @ W_emb + b_emb                      [b,t,N,D]
  send = relu(relu(emb@Ws1+bs1)@Ws2+bs2)             [b,t,N,D]
  recv = relu(relu(emb@Wr1+br1)@Wr2+br2)             [b,t,N,D]
  full = [send[:,1:,se]|pe[1:]|recv[:,:-1,re]|pe[:-1]]   [b,t-1,E,288]
  out  = relu(full@Wc1+bc1)@Wc2 + bc2                [b,t-1,E,5]

Factorizations:
 - The edge gather commutes with the (linear) first combine layer: per-node
   tables Xs[t] = send[t]@Wc1[0:128,:], Xr[t] = recv[t]@Wc1[144:272,:]
   (N=128 rows instead of E=1024). The gather + send/recv add runs as
   one-hot gather-matmuls accumulated in PSUM.
 - W_emb is folded into the first MLP layer (no activation in between):
   s1 = spk @ (W_emb@Ws1) etc., killing the embed matmul + PSUM drain.
 - The positional-encoding/bias term pe[t+1]@Wc1[128:144] + pe[t]@Wc1[272:288]
   + bc1 depends only on (t, feature) = the PSUM partition of the gathered
   pre-activations, so it is applied for free as the per-partition bias of
   the ReLU on the Scalar/Vector engines.
 - Stage B packs 4 output timesteps x 32 features per 128-partition tile
   (indices are t-independent, so one gather matmul serves 4 timesteps'
   feature chunks), and the output layer uses a block-diagonal [128,20]
   stationary producing 4 timesteps' [5,E] outputs per moving pass: both
   stages hit the PE moving-row floor (no partial-width stationaries).

All heavy matmuls run in bf16; PSUM accumulation stays fp32.

Sharding: 8 cores = 2 batches x 4 time chunks. Each core computes 64
output timesteps (chunk starts [0,64,128,191]; the last chunk overlaps the
third by one step so all cores run an identical program).
"""

import os
import sys

import numpy as np

sys.path.insert(0, "/opt/trn_rl_repo")

import concourse.bass as bass  # noqa: E402
import concourse.mybir as mybir  # noqa: E402
import concourse.tile as tile  # noqa: E402
from concourse.bass_utils import run_bass_kernel_spmd  # noqa: E402

B, T, N, F = 2, 256, 128, 16
D, H, E = 128, 288, 1024
PE_SIZE = 16
NCORES = 8
TCHUNK = 64            # output timesteps per core
TSTEPS = TCHUNK + 1    # node-level timesteps per core
ROWS = TSTEPS * N      # node-level rows per core (8320)
T_LOS = [0, 64, 128, 191]
NG = TCHUNK // 4       # stage-B groups of 4 output timesteps
NJ = H // 32           # feature chunks of 32 (9)
EC = 512               # edge chunk
NEC = E // EC

F32 = mybir.dt.float32
BF16 = mybir.dt.bfloat16

LAST_RESULTS = None    # BassKernelResults of the last run (for test harness)

_PROGRAM = None


def _build_program():
    nc = bass.Bass()

    def inp(name, shape):
        return nc.dram_tensor(name, shape, F32, kind="ExternalInput")

    def binp(name, shape):
        return nc.dram_tensor(name, shape, BF16, kind="ExternalInput")

    spk = binp("spk", [128, ROWS])       # spikes, feature-major, zero-padded
    # to 128 partitions so the first MLP matmul runs as a full (128,128) PE
    # tile (16-row tiles pay the narrow-tile mode-switch penalty)
    wse1 = binp("wse1", [128, D])        # W_emb @ Ws1, zero-padded rows
    bs1 = inp("bs1", [D, 1])             # bs1 + b_emb@Ws1 (per-partition col)
    ws2 = binp("ws2", [D, D])
    bs2 = inp("bs2", [D, 1])
    wre1 = binp("wre1", [128, D])        # W_emb @ Wr1, zero-padded rows
    br1 = inp("br1", [D, 1])
    wr2 = binp("wr2", [D, D])
    br2 = inp("br2", [D, 1])
    wc1s = binp("wc1s", [D, H])          # Wc1[0:128, :]
    wc1r = binp("wc1r", [D, H])          # Wc1[144:272, :]
    gs = binp("gs", [N, E])              # one-hot send gather matrix
    gr = binp("gr", [N, E])              # one-hot recv gather matrix
    petc = inp("petc", [128, NG * NJ])   # pre-ReLU bias col per (group, j)
    wc2b = binp("wc2b", [128, NJ, 128])  # block-diag Wc2 per j-chunk, zero-
    # padded to 128 output columns so the PE stays in (128,128) tile mode
    # (narrow-tile matmuls pay a ~100ns mode-switch penalty on themselves and
    # on the following matmul)

    outd = nc.dram_tensor("out", [NG, 20, E], F32, kind="ExternalOutput")

    relu = mybir.ActivationFunctionType.Relu
    alu_add = mybir.AluOpType.add
    alu_max = mybir.AluOpType.max

    with tile.TileContext(nc) as tc:
        with (
            tc.tile_pool(name="wpool", bufs=1) as wp,
            tc.tile_pool(name="mlp_ps", bufs=2, space="PSUM") as mlp_ps,
            tc.tile_pool(name="x_ps", bufs=2, space="PSUM") as x_ps,
            tc.tile_pool(name="pre_ps", bufs=3, space="PSUM") as pre_ps,
            tc.tile_pool(name="o_ps", bufs=1, space="PSUM") as o_ps,
            tc.tile_pool(name="sbA", bufs=3) as sbA,
            tc.tile_pool(name="sbB", bufs=6) as sbB,
        ):
            wse1_sb = wp.tile([128, D], BF16, tag="wse1")
            bs1_sb = wp.tile([D, 1], F32, tag="bs1")
            ws2_sb = wp.tile([D, D], BF16, tag="ws2")
            bs2_sb = wp.tile([D, 1], F32, tag="bs2")
            wre1_sb = wp.tile([128, D], BF16, tag="wre1")
            br1_sb = wp.tile([D, 1], F32, tag="br1")
            wr2_sb = wp.tile([D, D], BF16, tag="wr2")
            br2_sb = wp.tile([D, 1], F32, tag="br2")
            wc1s_sb = wp.tile([D, H], BF16, tag="wc1s")
            wc1r_sb = wp.tile([D, H], BF16, tag="wc1r")
            gs_sb = wp.tile([N, E], BF16, tag="gs")
            gr_sb = wp.tile([N, E], BF16, tag="gr")
            petc_sb = wp.tile([128, NG * NJ], F32, tag="petc")
            wc2b_sb = wp.tile([128, NJ, 128], BF16, tag="wc2b")
            # per-timestep node tables of the factored combine layer (bf16):
            # xs_tbl[:, j, t, :] = (send[t] @ Wc1[0:128])[:, 32j:32j+32]
            # xr_tbl[:, j, t, :] = (recv[t] @ Wc1[144:272])[:, 32j:32j+32]
            # j-major layout so a [4 timesteps x 32 features] stationary
            # slice is one contiguous 128-wide free dim (matmul stationary
            # APs allow only a single free dimension).
            xs_tbl = wp.tile([N, NJ, TSTEPS, 32], BF16, tag="xs_tbl")
            xr_tbl = wp.tile([N, NJ, TSTEPS, 32], BF16, tag="xr_tbl")

            def load_stage_a_weights():
                for sb_t, dr_t in [
                    (wse1_sb, wse1), (bs1_sb, bs1), (ws2_sb, ws2),
                    (bs2_sb, bs2), (wre1_sb, wre1), (br1_sb, br1),
                    (wr2_sb, wr2), (br2_sb, br2),
                    (wc1s_sb, wc1s), (wc1r_sb, wc1r),
                ]:
                    nc.sync.dma_start(sb_t[:], dr_t[:])

            def load_stage_b_weights():
                # issued after the first chunks' spk DMAs so the gather/output
                # weights don't delay the PE's first matmul
                for sb_t, dr_t in [
                    (gs_sb, gs), (gr_sb, gr), (petc_sb, petc), (wc2b_sb, wc2b),
                ]:
                    nc.sync.dma_start(sb_t[:], dr_t[:])

            def spk_dma(c):
                r0 = c * 512
                ch = min(512, ROWS - r0)
                t_ = sbA.tile([128, ch], BF16, tag="spk", name=f"spk{c}")
                nc.sync.dma_start(t_[:], spk[:, r0:r0 + ch])
                return t_

            mlp_out = {}

            def chunk_mlp(c, spk_pre=None):
                """Node MLPs for timesteps 4c..4c+3 (t=64: single timestep,
                send side only)."""
                r0 = c * 512
                ch = min(512, ROWS - r0)
                last = ch < 512
                spk_c = spk_pre if spk_pre is not None else spk_dma(c)

                s1_ps = mlp_ps.tile([D, ch], F32, tag="mlp", name=f"s1_{c}")
                nc.tensor.matmul(s1_ps[:], wse1_sb[:], spk_c[:])
                s1_sb = sbA.tile([D, ch], BF16, tag="s1", name=f"s1s{c}")
                nc.scalar.activation(s1_sb[:], s1_ps[:], relu,
                                     bias=bs1_sb[:, 0:1])
                if not last:
                    r1_ps = mlp_ps.tile([D, ch], F32, tag="mlp", name=f"r1_{c}")
                    nc.tensor.matmul(r1_ps[:], wre1_sb[:], spk_c[:])
                    r1_sb = sbA.tile([D, ch], BF16, tag="r1", name=f"r1s{c}")
                    nc.scalar.activation(r1_sb[:], r1_ps[:], relu,
                                         bias=br1_sb[:, 0:1])
                s2_ps = mlp_ps.tile([D, ch], F32, tag="mlp", name=f"s2_{c}")
                nc.tensor.matmul(s2_ps[:], ws2_sb[:], s1_sb[:])
                send_c = sbA.tile([D, ch], BF16, tag="sc", name=f"sc{c}")
                nc.scalar.activation(send_c[:], s2_ps[:], relu,
                                     bias=bs2_sb[:, 0:1])
                if not last:
                    r2_ps = mlp_ps.tile([D, ch], F32, tag="mlp", name=f"r2_{c}")
                    nc.tensor.matmul(r2_ps[:], wr2_sb[:], r1_sb[:])
                    recv_c = sbA.tile([D, ch], BF16, tag="rc", name=f"rc{c}")
                    nc.scalar.activation(recv_c[:], r2_ps[:], relu,
                                         bias=br2_sb[:, 0:1])

                mlp_out[c] = (send_c, None if last else recv_c, ch, last)

            def chunk_tables(c):
                """Xs/Xr combine-layer tables for chunk c's timesteps;
                emitted well after chunk_mlp(c) so the stationary send_c/
                recv_c ReLUs are long done when the PE reaches these. Table
                drains alternate DVE (xs) / ACT (xr) so the PSUM ring frees
                at two-engine rate."""
                send_c, recv_c, ch, last = mlp_out.pop(c)
                for k in range(ch // N):
                    t = 4 * c + k
                    if t > 0:  # xs[0] is never gathered (send uses t+1)
                        xsp = x_ps.tile([N, H], F32, tag="x", name=f"xs{t}")
                        nc.tensor.matmul(
                            xsp[:], send_c[:, k * N:(k + 1) * N], wc1s_sb[:])
                        nc.vector.tensor_copy(xs_tbl[:, :, t, :], xsp[:])
                    if not last:  # xr[64] is never gathered (recv uses t)
                        xrp = x_ps.tile([N, H], F32, tag="x", name=f"xr{t}")
                        nc.tensor.matmul(
                            xrp[:], recv_c[:, k * N:(k + 1) * N], wc1r_sb[:])
                        nc.scalar.activation(
                            xr_tbl[:, :, t, :], xrp[:],
                            mybir.ActivationFunctionType.Copy)

            def group(g):
                """Gather + biased ReLU + output layer for output timesteps
                4g..4g+3 (partition dim packs 4 timesteps x 32 features)."""
                o_sb = sbB.tile([20, E], F32, tag="o_sb", name=f"osb{g}")
                for ec in range(NEC):
                    o_t = o_ps.tile([128, EC], F32, tag="o", name=f"o{g}_{ec}")
                    hs = []
                    def out_mm(j):
                        nc.tensor.matmul(o_t[:], wc2b_sb[:, j, :], hs[j][:],
                                         start=(j == 0), stop=(j == NJ - 1))
                    for j in range(NJ):
                        pre = pre_ps.tile([128, EC], F32, tag="pre",
                                          name=f"pre{g}_{ec}_{j}")
                        nc.tensor.matmul(
                            pre[:],
                            xs_tbl[:, j, 4 * g + 1:4 * g + 5, :],
                            gs_sb[:, ec * EC:(ec + 1) * EC],
                            start=True, stop=False)
                        nc.tensor.matmul(
                            pre[:],
                            xr_tbl[:, j, 4 * g:4 * g + 4, :],
                            gr_sb[:, ec * EC:(ec + 1) * EC],
                            start=False, stop=True)
                        h = sbB.tile([128, EC], BF16, tag="h",
                                     name=f"h{g}_{ec}_{j}")
                        col = g * NJ + j
                        if j % 2 == 0 and j > 0:
                            nc.vector.tensor_scalar(
                                h[:], pre[:], petc_sb[:, col:col + 1], 0.0,
                                alu_add, alu_max)
                        else:
                            nc.scalar.activation(h[:], pre[:], relu,
                                                 bias=petc_sb[:, col:col + 1])
                        hs.append(h)
                        # run the output matmul two j-steps behind the fills
                        # so its moving operand's ReLU (and the LDWEIGHTS
                        # carrying that wait) is done well before the PE
                        # reaches it.
                        if j >= 2:
                            out_mm(j - 2)
                    out_mm(NJ - 2)
                    out_mm(NJ - 1)
                    nc.vector.tensor_copy(
                        o_sb[:, ec * EC:(ec + 1) * EC], o_t[0:20, :])
                nc.sync.dma_start(outd[g, :, :], o_sb[:])

            # interleave: group g needs tables through chunk g+1; emitting
            # groups between chunks lets the PE-bound gather phases absorb
            # the DVE-bound table drains of stage A.
            # PE p-state warmup: the PE ramps 0.65->1.2->2.4 GHz over ~3us
            # of continuous execution, and the first real matmul can't start
            # until the spk/weight DMAs land (~10us). Burn the ramp on
            # scratch matmuls that depend only on a memset.
            warm_in = sbB.tile([128, EC], BF16, tag="h", name="warm_in")
            nc.vector.memset(warm_in[:], 0)
            for w in range(12):
                wps = pre_ps.tile([128, EC], F32, tag="pre", name=f"warm{w}")
                nc.tensor.matmul(wps[:], warm_in[:, 0:128], warm_in[:])

            spk0 = spk_dma(0)
            load_stage_a_weights()
            chunk_mlp(0, spk_pre=spk0)
            load_stage_b_weights()
            chunk_mlp(1)
            chunk_mlp(2)
            chunk_tables(0)
            chunk_tables(1)
            group(0)
            for g in range(1, NG):
                if g + 2 <= 16:
                    chunk_mlp(g + 2)
                chunk_tables(g + 1)
                group(g)

    _legalize_waits(nc)
    return nc


def _legalize_waits(nc):
    """Walrus codegen rejects instructions carrying more than one sync wait
    ("Too many sync wait commands", CoreV3GenImpl setupSyncWait). Hoist all
    but the last wait of any instruction onto standalone InstEventSemaphore
    instructions inserted just before it on the same engine queue —
    semantically identical, since waits execute in program order."""
    for f in nc.m.functions:
        for blk in f.blocks:
            insts = blk.instructions
            if not any(
                i.sync_info is not None and len(i.sync_info.on_wait or ()) > 1
                for i in insts
            ):
                continue
            out = []
            for inst in insts:
                si = inst.sync_info
                waits = list(si.on_wait) if si is not None and si.on_wait else []
                if len(waits) > 1:
                    for w in waits[:-1]:
                        out.append(mybir.InstEventSemaphore(
                            name=nc.get_next_instruction_name(),
                            engine=inst.engine,
                            ins=[],
                            outs=[],
                            sync_info=mybir.SyncInfo(on_wait=[w], on_update=[]),
                        ))
                    si.on_wait = waits[-1:]
                out.append(inst)
            blk.instructions = out


def _get_program():
    global _PROGRAM
    if _PROGRAM is None:
        _PROGRAM = _build_program()
    return _PROGRAM


def _sinusoidal_pe(d, t):
    pos = np.arange(t, dtype=np.float32)[:, None]
    div = np.exp(np.arange(0, d, 2, dtype=np.float32)
                 * (-np.log(10000.0) / d)).astype(np.float32)
    pe = np.zeros((t, d), dtype=np.float32)
    pe[:, 0::2] = np.sin(pos * div)
    pe[:, 1::2] = np.cos(pos * div)
    return pe


def kernel(spikes, W_emb, b_emb, Ws1, bs1, Ws2, bs2, Wr1, br1, Wr2, br2,
           Wc1, bc1, Wc2, bc2, send_edges, recv_edges):
    global LAST_RESULTS
    f32 = np.float32
    spikes = np.asarray(spikes, f32)
    W_emb = np.asarray(W_emb, f32)
    Ws1 = np.asarray(Ws1, f32)
    Wr1 = np.asarray(Wr1, f32)
    Wc1 = np.asarray(Wc1, f32)
    Wc2 = np.asarray(Wc2, f32)
    se = np.asarray(send_edges).astype(np.int64)
    re_ = np.asarray(recv_edges).astype(np.int64)

    # Positional-encoding contribution to the pre-ReLU combine activations:
    # pet_full[t_out] = pe[t_out+1] @ Wc1[128:144] + pe[t_out] @ Wc1[272:288]
    #                   + bc1, shape [T-1, 288].
    pe = _sinusoidal_pe(PE_SIZE, T)
    pet_full = (pe[1:] @ Wc1[D:D + PE_SIZE]
                + pe[:-1] @ Wc1[D + PE_SIZE + D:]
                + np.asarray(bc1, f32)[None, :]).astype(f32)

    nodes = np.arange(N, dtype=np.int64)
    G_send = (se[None, :] == nodes[:, None]).astype(f32)        # [N, E]
    G_recv = (re_[None, :] == nodes[:, None]).astype(f32)       # [N, E]

    import ml_dtypes
    bf16 = ml_dtypes.bfloat16
    # fold the (activation-free) embed layer into the first MLP layer:
    # ((spk@W_emb)+b_emb) @ W + b == spk @ (W_emb@W) + (b + b_emb@W)
    b_emb_v = np.asarray(b_emb, f32).reshape(1, D)
    bs1_f = np.asarray(bs1, f32) + (b_emb_v @ Ws1)[0]
    br1_f = np.asarray(br1, f32) + (b_emb_v @ Wr1)[0]

    # block-diagonal output-layer stationaries: wc2b[:, j, :] maps the
    # (4 timesteps x 32 features) partition layout to 4 timesteps x 5 outputs
    wc2b = np.zeros((128, NJ, 128), f32)
    for j in range(NJ):
        blk = Wc2[32 * j:32 * j + 32]                           # [32, 5]
        for jj in range(4):
            wc2b[32 * jj:32 * jj + 32, j, 5 * jj:5 * jj + 5] = blk

    def pad128(w):
        out = np.zeros((128, w.shape[1]), f32)
        out[:w.shape[0]] = w
        return out

    common = dict(
        wse1=pad128(W_emb @ Ws1).astype(bf16),
        bs1=np.ascontiguousarray(bs1_f.reshape(D, 1)),
        ws2=np.ascontiguousarray(np.asarray(Ws2, f32)).astype(bf16),
        bs2=np.ascontiguousarray(np.asarray(bs2, f32).reshape(D, 1)),
        wre1=pad128(W_emb @ Wr1).astype(bf16),
        br1=np.ascontiguousarray(br1_f.reshape(D, 1)),
        wr2=np.ascontiguousarray(np.asarray(Wr2, f32)).astype(bf16),
        br2=np.ascontiguousarray(np.asarray(br2, f32).reshape(D, 1)),
        wc1s=np.ascontiguousarray(Wc1[0:D]).astype(bf16),
        wc1r=np.ascontiguousarray(Wc1[D + PE_SIZE:D + PE_SIZE + D]).astype(bf16),
        gs=G_send.astype(bf16),
        gr=G_recv.astype(bf16),
        wc2b=wc2b.astype(bf16),
    )

    in_maps = []
    for core in range(NCORES):
        b = core // 4
        t_lo = T_LOS[core % 4]
        spk_slice = spikes[b, t_lo:t_lo + TSTEPS]               # [65,128,16]
        spkT = np.zeros((128, ROWS), f32)
        spkT[:F] = spk_slice.reshape(ROWS, F).T
        spkT = spkT.astype(bf16)                                # [128, 8320]
        # per-partition pre-ReLU bias columns: petc[32*jj+c, g*NJ+j] =
        # pet_full[t_lo + 4g + jj, 32j + c]
        pf = pet_full[t_lo:t_lo + TCHUNK].reshape(NG, 4, NJ, 32)
        petc = np.ascontiguousarray(
            pf.transpose(1, 3, 0, 2).reshape(128, NG * NJ))
        in_maps.append(dict(common, spk=spkT, petc=petc))

    nc = _get_program()
    trace = bool(int(os.environ.get("KERNEL_TRACE", "0")))
    res = run_bass_kernel_spmd(nc, in_maps, list(range(NCORES)), trace=trace)
    LAST_RESULTS = res

    out = np.zeros((B, T - 1, E, 5), f32)
    for core in range(NCORES):
        b = core // 4
        t_lo = T_LOS[core % 4]
        r = res.results[core]["out"]                            # [16, 20, 1024]
        r = r.reshape(NG, 4, 5, E).reshape(TCHUNK, 5, E)
        out[b, t_lo:t_lo + TCHUNK] = r.transpose(0, 2, 1)
    out += np.asarray(bc2, f32)[None, None, None, :]
    return out
